# revision 1
# baseline (speedup 1.0000x reference)
"""GCNAggregator Trainium2 Bass kernel.

out[i] = (sum_{e: seg[e]==i} features[neighbor_idx[e]] + features[i]) / (deg_i + 1)

Strategy (8 NeuronCores, SPMD):
  - Nodes are sharded into 8 contiguous, edge-balanced ranges (<=6272 nodes
    each). Since segment_ids is sorted, each core's incident edges are a
    contiguous range of the edge list.
  - Per core, nodes are packed greedily into "slots" of <=128 consecutive
    nodes, capped by per-slot edge counts so every slot is nearly full of
    edges. Slot tile counts are maxed over the 8 cores so the compiled
    program is identical on every core (SPMD) at ~2% gather padding.
  - Features are stored as an interleaved hi/lo bf16 table (hi = bf16(x),
    lo = bf16(x - hi); still 1KB/row, so gather traffic is unchanged) and
    each slot's edges are gathered from HBM with gpsimd.dma_gather, then
    segment-summed on the tensor engine via one 512-wide one-hot matmul
    per 128-edge tile:
        psum[128 nodes, 512] += onehot[128 edges, 128 nodes]^T @ G[128 edges, 512]
    The one-hot is exact in bf16 and PSUM accumulates in fp32, so the sum
    keeps ~18 mantissa bits (end-to-end rel err ~3e-6) at 1 cycle/row
    matmul throughput instead of fp32's 4 cycles/row. The one-hot is built
    on device from per-edge relative segment ids (is_equal vs an iota row).
  - dma_gather indices are int16 (max 32767) but the table has 50000 rows,
    so each group's edges are split into a low class (row < 32768) and a
    high class (row >= 32768, gathered from an offset view of the table).
  - A single dma_gather is limited to 1024 indices (SWDGE descriptor ring
    capacity), so each class run is issued as <=1024-row gather calls into
    column slices of one per-group SBUF tile.
  - Finalize per slot: out = (psum + features[self]) * 1/(deg+1), DMA out.

The host only computes integer index metadata (shard boundaries, per-slot
class-split index streams, relative segment ids, degrees); all floating
point work (gather, segment sum, self-add, normalize) runs on device.
"""

import sys

import numpy as np

try:
    import concourse  # noqa: F401
except ImportError:  # pragma: no cover
    sys.path.insert(0, "/opt/trn_rl_repo")

from contextlib import ExitStack

import concourse.mybir as mybir
from concourse import bacc, bass_utils, tile

N_NODES = 50000
N_EDGES = 1_600_000
D = 256
N_CORES = 8
NPC = 6272          # node slots per core (= GROUPS * 128)
GROUPS = 49
SPLIT = 32768       # int16 gather-index window
H_ROWS = N_NODES - SPLIT

_PROGRAM_CACHE: dict = {}
LAST_NC = None  # exposed for test harness introspection (TimelineSim)

MAX_GATHER = 1024  # SWDGE ring capacity per dma_gather call


def _chunks(total_rows):
    out = []
    off = 0
    while off < total_rows:
        k = min(MAX_GATHER, total_rows - off)
        out.append((off, k))
        off += k
    return out


def _build_program(t_l_arr: tuple, t_h_arr: tuple):
    """Build + compile the (uniform-across-cores, SPMD) per-core program.

    t_l_arr/t_h_arr: per-group tile counts (max over the 8 cores), so the
    program structure is identical on every core while padding stays low.
    """
    n_slots = len(t_l_arr)
    tiles_g = [t_l_arr[g] + t_h_arr[g] for g in range(n_slots)]
    nt_tot = sum(tiles_g)
    rows_tot = nt_tot * 128
    ni16 = rows_tot // 16  # gidx columns (wrapped-16 int16 layout)
    # column offset of each group's tile block
    m_off = np.concatenate([[0], np.cumsum(tiles_g)]).astype(int)

    nc = bacc.Bacc(
        "TRN2", target_bir_lowering=False, debug=False, num_devices=N_CORES
    )

    # hi/lo bf16 split table: row i = [bf16(x_i) | bf16(x_i - hi_i)], 1KB/row.
    # One-hot matmuls against each half are exact in bf16 and accumulate in
    # fp32 PSUM, so the segment sum keeps ~18 mantissa bits at 1 cycle/row
    # matmul throughput (vs 4 cycles/row for fp32 operands).
    feat_d = nc.dram_tensor(
        "features2", (N_NODES, 2 * D), mybir.dt.bfloat16, kind="ExternalInput"
    ).ap()
    gidx_d = nc.dram_tensor(
        "gidx", (128, ni16), mybir.dt.int16, kind="ExternalInput"
    ).ap()
    srel_d = nc.dram_tensor(
        "srel", (128, nt_tot), mybir.dt.float32, kind="ExternalInput"
    ).ap()
    cnt1_d = nc.dram_tensor(
        "cnt1", (128, n_slots), mybir.dt.float32, kind="ExternalInput"
    ).ap()
    fself_d = nc.dram_tensor(
        "fself", (n_slots * 128, D), mybir.dt.float32, kind="ExternalInput"
    ).ap()
    out_d = nc.dram_tensor(
        "out", (n_slots * 128, D), mybir.dt.float32, kind="ExternalOutput"
    ).ap()

    feat_lo = feat_d[0:SPLIT, :]
    feat_hi = feat_d[SPLIT:N_NODES, :]

    with tile.TileContext(nc) as tc:
        with ExitStack() as ctx:
            import os

            gb = int(os.environ.get("GT_BUFS", "2"))
            ob = int(os.environ.get("OH_BUFS", "4"))
            fb = int(os.environ.get("FIN_BUFS", "3"))
            pb = int(os.environ.get("PSUM_BUFS", "2"))
            const_pool = ctx.enter_context(tc.tile_pool(name="const", bufs=1))
            g_pool = ctx.enter_context(tc.tile_pool(name="gt", bufs=gb))
            oh_pool = ctx.enter_context(tc.tile_pool(name="oh", bufs=ob))
            fin_pool = ctx.enter_context(tc.tile_pool(name="fin", bufs=fb))
            psum_pool = ctx.enter_context(
                tc.tile_pool(name="psum", bufs=pb, space="PSUM")
            )

            # persistent metadata in SBUF (gidx loaded in chunks so early
            # gathers don't wait on the full 3.4MB index transfer)
            gidx_sb = const_pool.tile([128, ni16], mybir.dt.int16)
            n_ld = 8
            ld_bounds = [ni16 * i // n_ld for i in range(n_ld + 1)]
            for a, b in zip(ld_bounds[:-1], ld_bounds[1:]):
                if b > a:
                    nc.sync.dma_start(gidx_sb[:, a:b], gidx_d[:, a:b])
            srel_sb = const_pool.tile([128, nt_tot], mybir.dt.float32)
            nc.sync.dma_start(srel_sb[:], srel_d[:])
            cnt1_sb = const_pool.tile([128, n_slots], mybir.dt.float32)
            nc.sync.dma_start(cnt1_sb[:], cnt1_d[:])

            iota_i = const_pool.tile([128, 128], mybir.dt.int32)
            nc.gpsimd.iota(iota_i[:], pattern=[[1, 128]], base=0, channel_multiplier=0)
            iota_f = const_pool.tile([128, 128], mybir.dt.float32)
            nc.vector.tensor_copy(iota_f[:], iota_i[:])

            max_tiles = max(tiles_g)
            for g in range(n_slots):
                t_l = t_l_arr[g]
                n_tiles = tiles_g[g]
                m0 = int(m_off[g])
                c0 = m0 * 8  # 128 rows -> 8 int16-wrapped columns
                gt = g_pool.tile(
                    [128, max_tiles, 2 * D], mybir.dt.bfloat16, tag="gt"
                )
                for off, k in _chunks(t_l * 128):
                    nc.gpsimd.dma_gather(
                        gt[:, off // 128 : (off + k) // 128, :], feat_lo,
                        gidx_sb[:, c0 + off // 16 : c0 + (off + k) // 16],
                        num_idxs=k, num_idxs_reg=k,
                        elem_size=2 * D, elem_step=2 * D,
                    )
                for off, k in _chunks(t_h_arr[g] * 128):
                    nc.gpsimd.dma_gather(
                        gt[:, t_l + off // 128 : t_l + (off + k) // 128, :],
                        feat_hi,
                        gidx_sb[
                            :,
                            c0 + t_l * 8 + off // 16 : c0 + t_l * 8 + (off + k) // 16,
                        ],
                        num_idxs=k, num_idxs_reg=k,
                        elem_size=2 * D, elem_step=2 * D,
                    )

                # one 512-wide matmul per tile fills a full PSUM bank:
                # psum[:, 0:D] accumulates the hi halves, psum[:, D:2D] the
                # lo halves; they are summed at finalize.
                psum = psum_pool.tile([128, 2 * D], mybir.dt.float32, tag="ps")
                for t in range(n_tiles):
                    oh = oh_pool.tile([128, 128], mybir.dt.bfloat16, tag="oh")
                    nc.vector.tensor_scalar(
                        oh[:], iota_f[:], srel_sb[:, m0 + t : m0 + t + 1], None,
                        op0=mybir.AluOpType.is_equal,
                    )
                    nc.tensor.matmul(
                        psum[:], oh[:], gt[:, t, :],
                        start=(t == 0), stop=(t == n_tiles - 1),
                    )

                fs = fin_pool.tile([128, D], mybir.dt.float32, tag="fs")
                nc.sync.dma_start(fs[:], fself_d[g * 128 : (g + 1) * 128, :])
                rec = fin_pool.tile([128, 1], mybir.dt.float32, tag="rec")
                nc.vector.reciprocal(rec[:], cnt1_sb[:, g : g + 1])
                o_sb = fin_pool.tile([128, D], mybir.dt.float32, tag="o")
                # DVE may read at most one PSUM operand per instruction
                nc.vector.tensor_add(o_sb[:], psum[:, 0:D], fs[:])
                nc.vector.tensor_add(o_sb[:], o_sb[:], psum[:, D : 2 * D])
                nc.vector.tensor_scalar_mul(o_sb[:], o_sb[:], rec[:])
                nc.sync.dma_start(out_d[g * 128 : (g + 1) * 128, :], o_sb[:])

    nc.compile()
    return nc


def _pack_slots(cum_l, cum_h, n_nodes, cap_l, cap_h):
    """Greedy variable-width node slots: each slot takes consecutive nodes
    (<=128) while its L/H edge counts stay under the caps. Returns a list of
    (base, width, nL, nH)."""
    slots = []
    i = 0
    while i < n_nodes:
        jmax = min(i + 128, n_nodes)
        jl = int(np.searchsorted(cum_l, cum_l[i] + cap_l * 128, side="right")) - 1
        jh = int(np.searchsorted(cum_h, cum_h[i] + cap_h * 128, side="right")) - 1
        j = max(min(jmax, jl, jh), i + 1)
        slots.append(
            (i, j - i, int(cum_l[j] - cum_l[i]), int(cum_h[j] - cum_h[i]))
        )
        i = j
    return slots


def _preprocess(features, neighbor_idx, segment_ids):
    """Host-side shard/index metadata construction (integers only)."""
    feat = np.ascontiguousarray(np.asarray(features, dtype=np.float32))
    seg = np.asarray(segment_ids).astype(np.int64)
    nid = np.asarray(neighbor_idx).astype(np.int64)
    n_edges = seg.shape[0]

    # interleaved hi/lo bf16 gather table (exact two-term bf16 decomposition)
    bf16 = mybir.dt.np(mybir.dt.bfloat16)
    feat2 = np.empty((N_NODES, 2 * D), bf16)
    hi = feat.astype(bf16)
    feat2[:, :D] = hi
    feat2[:, D:] = (feat - hi.astype(np.float32)).astype(bf16)

    deg = np.bincount(seg, minlength=N_NODES)

    # edge-balanced core node boundaries (spans capped at NPC node slots)
    bounds = [0]
    for c in range(1, N_CORES):
        n = int(seg[min(c * n_edges // N_CORES, n_edges - 1)])
        n = min(n, bounds[-1] + NPC)
        n = max(n, N_NODES - (N_CORES - c) * NPC, bounds[-1])
        bounds.append(n)
    bounds.append(N_NODES)

    # per-core edge slices and per-node class-split prefix sums
    per_core = []
    for c in range(N_CORES):
        lo, hi = np.searchsorted(seg, [bounds[c], bounds[c + 1]])
        s = seg[lo:hi] - bounds[c]
        x = nid[lo:hi]
        nn = bounds[c + 1] - bounds[c]
        is_l = x < SPLIT
        cnt_l = np.bincount(s[is_l], minlength=nn)
        cnt_h = np.bincount(s[~is_l], minlength=nn)
        cum_l = np.concatenate([[0], np.cumsum(cnt_l)])
        cum_h = np.concatenate([[0], np.cumsum(cnt_h)])
        per_core.append((s, x, nn, cum_l, cum_h))

    # choose caps minimizing total (uniform-across-cores) tile count
    best = None
    for cap_l in range(17, 24):
        for cap_h in range(9, 13):
            all_slots = [
                _pack_slots(pc[3], pc[4], pc[2], cap_l, cap_h) for pc in per_core
            ]
            n_slots = max(len(sl) for sl in all_slots)
            tl = np.zeros(n_slots, np.int64)
            th = np.zeros(n_slots, np.int64)
            for sl in all_slots:
                for g, (_, _, nl, nh) in enumerate(sl):
                    tl[g] = max(tl[g], -(-nl // 128))
                    th[g] = max(th[g], -(-nh // 128))
            total = int(tl.sum() + th.sum())
            if best is None or total < best[0]:
                best = (total, tuple(int(v) for v in tl), tuple(int(v) for v in th), all_slots)
    _, t_l_arr, t_h_arr, all_slots = best
    # a slot with zero tiles would leave its PSUM accumulator unwritten
    t_l_arr = tuple(
        max(tl, 1) if tl + th == 0 else tl for tl, th in zip(t_l_arr, t_h_arr)
    )
    n_slots = len(t_l_arr)

    tiles_g = [t_l_arr[g] + t_h_arr[g] for g in range(n_slots)]
    nt_tot = sum(tiles_g)
    m_off = np.concatenate([[0], np.cumsum(tiles_g)]).astype(int)

    in_maps = []
    slot_maps = []
    for c in range(N_CORES):
        s, x, nn, _, _ = per_core[c]
        slots = all_slots[c]
        gidx_all = np.zeros(nt_tot * 128, np.int16)
        srel_all = np.full((nt_tot, 128), -1.0, np.float32)
        cnt1 = np.ones((128, n_slots), np.float32)
        fself = np.zeros((n_slots * 128, D), np.float32)
        node_bnds = [sl[0] for sl in slots] + [nn]
        edge_bnds = np.searchsorted(s, node_bnds)
        for g, (base_n, width, _, _) in enumerate(slots):
            t_l, t_h = t_l_arr[g], t_h_arr[g]
            kl, kh = t_l * 128, t_h * 128
            a, b = edge_bnds[g], edge_bnds[g + 1]
            sg = s[a:b]
            xg = x[a:b]
            m = xg < SPLIT
            xl = xg[m]
            xh = xg[~m] - SPLIT
            sl_ = sg[m] - base_n
            sh_ = sg[~m] - base_n
            # sort each run by source row: the one-hot matmul is order-
            # invariant within a slot, and address-sorted gather descriptors
            # get HBM row-buffer locality (duplicates become adjacent)
            ol = np.argsort(xl, kind="stable")
            xl, sl_ = xl[ol], sl_[ol]
            oh_ = np.argsort(xh, kind="stable")
            xh, sh_ = xh[oh_], sh_[oh_]
            m0 = int(m_off[g])
            base = m0 * 128
            gidx_all[base : base + len(xl)] = xl.astype(np.int16)
            gidx_all[base + kl : base + kl + len(xh)] = xh.astype(np.int16)
            srl = np.full(kl, -1.0, np.float32)
            srl[: len(sl_)] = sl_
            srel_all[m0 : m0 + t_l] = srl.reshape(t_l, 128)
            srh = np.full(kh, -1.0, np.float32)
            srh[: len(sh_)] = sh_
            srel_all[m0 + t_l : m0 + t_l + t_h] = srh.reshape(t_h, 128)
            abs_base = bounds[c] + base_n
            cnt1[:width, g] = 1.0 + deg[abs_base : abs_base + width]
            fself[g * 128 : g * 128 + width] = feat[abs_base : abs_base + width]

        gidx_w = np.ascontiguousarray(np.tile(gidx_all.reshape(-1, 16).T, (8, 1)))
        srel_mat = np.ascontiguousarray(srel_all.T)
        in_maps.append(
            {
                "features2": feat2,
                "gidx": gidx_w,
                "srel": srel_mat,
                "cnt1": cnt1,
                "fself": fself,
            }
        )
        slot_maps.append(
            [(bounds[c] + sl[0], sl[1]) for sl in slots]
        )
    return t_l_arr, t_h_arr, in_maps, slot_maps


def kernel(features, neighbor_idx, segment_ids):
    global LAST_NC
    t_l_arr, t_h_arr, in_maps, slot_maps = _preprocess(
        features, neighbor_idx, segment_ids
    )

    key = (t_l_arr, t_h_arr)
    if key not in _PROGRAM_CACHE:
        _PROGRAM_CACHE[key] = _build_program(t_l_arr, t_h_arr)
    nc = _PROGRAM_CACHE[key]
    LAST_NC = nc

    try:
        res = bass_utils.run_bass_kernel_spmd(
            nc, in_maps, core_ids=list(range(N_CORES))
        )
    except Exception:
        # transient axon/device hiccups (e.g. recovering from a prior wedge)
        # have been observed to clear after a short pause
        import time

        time.sleep(20)
        res = bass_utils.run_bass_kernel_spmd(
            nc, in_maps, core_ids=list(range(N_CORES))
        )

    out = np.empty((N_NODES, D), np.float32)
    for c in range(N_CORES):
        oc = res.results[c]["out"]
        for g, (abs_base, width) in enumerate(slot_maps[c]):
            out[abs_base : abs_base + width] = oc[g * 128 : g * 128 + width]
    return out



# revision 4
# speedup vs baseline: 1.7913x; 1.7913x over previous
"""GCNAggregator Trainium2 Bass kernel.

out[i] = (sum_{e: seg[e]==i} features[neighbor_idx[e]] + features[i]) / (deg_i + 1)

Strategy (8 NeuronCores, SPMD):
  - Nodes are sharded into 8 contiguous, edge-balanced ranges (<=6272 nodes
    each). Since segment_ids is sorted, each core's incident edges are a
    contiguous range of the edge list. The self-loop is folded in as one
    extra edge per node, so the whole aggregation is a single segment-sum.
  - Per core, nodes are packed greedily into "slots" of <=128 consecutive
    nodes, capped by per-slot edge counts so every slot is nearly full of
    edges. Slot tile counts are maxed over the 8 cores so the compiled
    program is identical on every core (SPMD) at ~2% gather padding.
  - Features are stored as a bf16 table (512B/row). Each slot's edges are
    gathered from HBM with gpsimd.dma_gather, then segment-summed on the
    tensor engine via one 256-wide one-hot matmul per 128-edge tile:
        psum[128 nodes, 256] += onehot[128 edges, 128 nodes]^T @ G[128 edges, 256]
    The one-hot is exact in bf16 and PSUM accumulates in fp32 (end-to-end
    rel err ~3e-3, well inside the 2e-2 gate) at 1 cycle/row matmul
    throughput. The one-hot is built on device from per-edge relative
    segment ids (is_equal vs an iota row, all-bf16 for 2x DVE throughput).
  - dma_gather indices are int16 (max 32767) but the table has 50000 rows,
    so each slot's edges are split into a low class (row < 32768) and a
    high class (row >= 32768, gathered from an offset view of the table).
  - The SWDGE descriptor ring is enlarged (dynamic_dma_scratch_size=65536
    -> 4096 descriptors) so each class run is one or two dma_gather calls
    per slot, amortizing the ~1us fixed descriptor-generation cost on the
    Pool engine.
  - Finalize per slot: out = psum * 1/(deg+1) (bf16), DMA out.

The host only computes integer index metadata (shard boundaries, per-slot
class-split index streams, relative segment ids, degrees); all floating
point work (gather, segment sum, normalize) runs on device.
"""

import sys

import numpy as np

try:
    import concourse  # noqa: F401
except ImportError:  # pragma: no cover
    sys.path.insert(0, "/opt/trn_rl_repo")

from contextlib import ExitStack

import concourse.mybir as mybir
from concourse import bacc, bass_utils, tile

N_NODES = 50000
N_EDGES = 1_600_000
D = 256
N_CORES = 8
NPC = 6272          # node slots per core (= GROUPS * 128)
GROUPS = 49
SPLIT = 32768       # int16 gather-index window

_PROGRAM_CACHE: dict = {}
LAST_NC = None  # exposed for test harness introspection (TimelineSim)

import os as _os

DMA_SCRATCH = int(_os.environ.get("DMA_SCRATCH", "16384"))
MAX_GATHER = DMA_SCRATCH // 16  # SWDGE descriptor ring capacity per call


def _chunks(total_rows):
    out = []
    off = 0
    while off < total_rows:
        k = min(MAX_GATHER, total_rows - off)
        out.append((off, k))
        off += k
    return out


def _build_program(t_l_arr: tuple, t_h_arr: tuple):
    """Build + compile the (uniform-across-cores, SPMD) per-core program.

    t_l_arr/t_h_arr: per-group tile counts (max over the 8 cores), so the
    program structure is identical on every core while padding stays low.
    """
    n_slots = len(t_l_arr)
    tiles_g = [t_l_arr[g] + t_h_arr[g] for g in range(n_slots)]
    nt_tot = sum(tiles_g)
    rows_tot = nt_tot * 128
    ni16 = rows_tot // 16  # gidx columns (wrapped-16 int16 layout)
    # column offset of each group's tile block
    m_off = np.concatenate([[0], np.cumsum(tiles_g)]).astype(int)

    nc = bacc.Bacc(
        "TRN2", target_bir_lowering=False, debug=False, num_devices=N_CORES,
        dynamic_dma_scratch_size=DMA_SCRATCH,
    )

    feat_d = nc.dram_tensor(
        "featb", (N_NODES, D), mybir.dt.bfloat16, kind="ExternalInput"
    ).ap()
    gidx_d = nc.dram_tensor(
        "gidx", (128, ni16), mybir.dt.int16, kind="ExternalInput"
    ).ap()
    srel_d = nc.dram_tensor(
        "srel", (128, nt_tot), mybir.dt.float32, kind="ExternalInput"
    ).ap()
    cnt1_d = nc.dram_tensor(
        "cnt1", (128, n_slots), mybir.dt.float32, kind="ExternalInput"
    ).ap()
    out_d = nc.dram_tensor(
        "out", (n_slots * 128, D), mybir.dt.bfloat16, kind="ExternalOutput"
    ).ap()

    feat_lo = feat_d[0:SPLIT, :]
    feat_hi = feat_d[SPLIT:N_NODES, :]

    with tile.TileContext(nc) as tc:
        with ExitStack() as ctx:
            import os

            gb = int(os.environ.get("GT_BUFS", "2"))
            ob = int(os.environ.get("OH_BUFS", "4"))
            fb = int(os.environ.get("FIN_BUFS", "3"))
            pb = int(os.environ.get("PSUM_BUFS", "4"))
            const_pool = ctx.enter_context(tc.tile_pool(name="const", bufs=1))
            g_pool = ctx.enter_context(tc.tile_pool(name="gt", bufs=gb))
            oh_pool = ctx.enter_context(tc.tile_pool(name="oh", bufs=ob))
            fin_pool = ctx.enter_context(tc.tile_pool(name="fin", bufs=fb))
            psum_pool = ctx.enter_context(
                tc.tile_pool(name="psum", bufs=pb, space="PSUM")
            )

            # persistent metadata in SBUF (gidx loaded in chunks so early
            # gathers don't wait on the full index transfer)
            gidx_sb = const_pool.tile([128, ni16], mybir.dt.int16)
            n_ld = 8
            ld_bounds = [ni16 * i // n_ld for i in range(n_ld + 1)]
            for a, b in zip(ld_bounds[:-1], ld_bounds[1:]):
                if b > a:
                    nc.sync.dma_start(gidx_sb[:, a:b], gidx_d[:, a:b])
            srel_sb = const_pool.tile([128, nt_tot], mybir.dt.float32)
            nc.sync.dma_start(srel_sb[:], srel_d[:])
            cnt1_sb = const_pool.tile([128, n_slots], mybir.dt.float32)
            nc.sync.dma_start(cnt1_sb[:], cnt1_d[:])

            iota_i = const_pool.tile([128, 128], mybir.dt.int32)
            nc.gpsimd.iota(iota_i[:], pattern=[[1, 128]], base=0, channel_multiplier=0)
            iota_f = const_pool.tile([128, 128], mybir.dt.bfloat16)
            nc.vector.tensor_copy(iota_f[:], iota_i[:])

            max_tiles = max(tiles_g)
            for g in range(n_slots):
                t_l = t_l_arr[g]
                n_tiles = tiles_g[g]
                m0 = int(m_off[g])
                c0 = m0 * 8  # 128 rows -> 8 int16-wrapped columns
                gt = g_pool.tile(
                    [128, max_tiles, D], mybir.dt.bfloat16, tag="gt"
                )
                for off, k in _chunks(t_l * 128):
                    nc.gpsimd.dma_gather(
                        gt[:, off // 128 : (off + k) // 128, :], feat_lo,
                        gidx_sb[:, c0 + off // 16 : c0 + (off + k) // 16],
                        num_idxs=k, num_idxs_reg=k,
                        elem_size=D, elem_step=D,
                    )
                for off, k in _chunks(t_h_arr[g] * 128):
                    nc.gpsimd.dma_gather(
                        gt[:, t_l + off // 128 : t_l + (off + k) // 128, :],
                        feat_hi,
                        gidx_sb[
                            :,
                            c0 + t_l * 8 + off // 16 : c0 + t_l * 8 + (off + k) // 16,
                        ],
                        num_idxs=k, num_idxs_reg=k,
                        elem_size=D, elem_step=D,
                    )

                psum = psum_pool.tile([128, D], mybir.dt.float32, tag="ps")
                for t in range(n_tiles):
                    oh = oh_pool.tile([128, 128], mybir.dt.bfloat16, tag="oh")
                    nc.vector.tensor_scalar(
                        oh[:], iota_f[:], srel_sb[:, m0 + t : m0 + t + 1], None,
                        op0=mybir.AluOpType.is_equal,
                    )
                    nc.tensor.matmul(
                        psum[:], oh[:], gt[:, t, :],
                        start=(t == 0), stop=(t == n_tiles - 1),
                    )

                rec = fin_pool.tile([128, 1], mybir.dt.float32, tag="rec")
                nc.vector.reciprocal(rec[:], cnt1_sb[:, g : g + 1])
                o_sb = fin_pool.tile([128, D], mybir.dt.bfloat16, tag="o")
                nc.vector.tensor_scalar_mul(o_sb[:], psum[:], rec[:])
                nc.sync.dma_start(out_d[g * 128 : (g + 1) * 128, :], o_sb[:])

    nc.compile()
    return nc


def _pack_slots(cum_l, cum_h, n_nodes, cap_l, cap_h):
    """Greedy variable-width node slots: each slot takes consecutive nodes
    (<=128) while its L/H edge counts stay under the caps. Returns a list of
    (base, width, nL, nH)."""
    slots = []
    i = 0
    while i < n_nodes:
        jmax = min(i + 128, n_nodes)
        jl = int(np.searchsorted(cum_l, cum_l[i] + cap_l * 128, side="right")) - 1
        jh = int(np.searchsorted(cum_h, cum_h[i] + cap_h * 128, side="right")) - 1
        j = max(min(jmax, jl, jh), i + 1)
        slots.append(
            (i, j - i, int(cum_l[j] - cum_l[i]), int(cum_h[j] - cum_h[i]))
        )
        i = j
    return slots


def _preprocess(features, neighbor_idx, segment_ids):
    """Host-side shard/index metadata construction (integers only)."""
    feat = np.ascontiguousarray(np.asarray(features, dtype=np.float32))
    seg = np.asarray(segment_ids).astype(np.int64)
    nid = np.asarray(neighbor_idx).astype(np.int64)
    n_edges = seg.shape[0]

    bf16 = mybir.dt.np(mybir.dt.bfloat16)
    featb = feat.astype(bf16)

    deg = np.bincount(seg, minlength=N_NODES)

    # edge-balanced core node boundaries (spans capped at NPC node slots)
    bounds = [0]
    for c in range(1, N_CORES):
        n = int(seg[min(c * n_edges // N_CORES, n_edges - 1)])
        n = min(n, bounds[-1] + NPC)
        n = max(n, N_NODES - (N_CORES - c) * NPC, bounds[-1])
        bounds.append(n)
    bounds.append(N_NODES)

    # per-core edge slices (self-loop folded in as one extra edge per node)
    # and per-node class-split prefix sums
    per_core = []
    for c in range(N_CORES):
        lo, hi = np.searchsorted(seg, [bounds[c], bounds[c + 1]])
        nn = bounds[c + 1] - bounds[c]
        s = np.concatenate([seg[lo:hi] - bounds[c], np.arange(nn)])
        x = np.concatenate([nid[lo:hi], np.arange(bounds[c], bounds[c + 1])])
        order = np.argsort(s, kind="stable")
        s = s[order]
        x = x[order]
        is_l = x < SPLIT
        cnt_l = np.bincount(s[is_l], minlength=nn)
        cnt_h = np.bincount(s[~is_l], minlength=nn)
        cum_l = np.concatenate([[0], np.cumsum(cnt_l)])
        cum_h = np.concatenate([[0], np.cumsum(cnt_h)])
        per_core.append((s, x, nn, cum_l, cum_h))

    # choose caps minimizing total (uniform-across-cores) tile count
    best = None
    for cap_l in range(17, 27):
        for cap_h in range(8, 15):
            all_slots = [
                _pack_slots(pc[3], pc[4], pc[2], cap_l, cap_h) for pc in per_core
            ]
            n_slots = max(len(sl) for sl in all_slots)
            tl = np.zeros(n_slots, np.int64)
            th = np.zeros(n_slots, np.int64)
            for sl in all_slots:
                for g, (_, _, nl, nh) in enumerate(sl):
                    tl[g] = max(tl[g], -(-nl // 128))
                    th[g] = max(th[g], -(-nh // 128))
            total = int(tl.sum() + th.sum())
            if best is None or total < best[0]:
                best = (total, tuple(int(v) for v in tl), tuple(int(v) for v in th), all_slots)
    _, t_l_arr, t_h_arr, all_slots = best
    # a slot with zero tiles would leave its PSUM accumulator unwritten
    t_l_arr = tuple(
        max(tl, 1) if tl + th == 0 else tl for tl, th in zip(t_l_arr, t_h_arr)
    )
    n_slots = len(t_l_arr)

    tiles_g = [t_l_arr[g] + t_h_arr[g] for g in range(n_slots)]
    nt_tot = sum(tiles_g)
    m_off = np.concatenate([[0], np.cumsum(tiles_g)]).astype(int)

    in_maps = []
    slot_maps = []
    for c in range(N_CORES):
        s, x, nn, _, _ = per_core[c]
        slots = all_slots[c]
        gidx_all = np.zeros(nt_tot * 128, np.int16)
        srel_all = np.full((nt_tot, 128), -1.0, np.float32)
        cnt1 = np.ones((128, n_slots), np.float32)
        node_bnds = [sl[0] for sl in slots] + [nn]
        edge_bnds = np.searchsorted(s, node_bnds)
        for g, (base_n, width, _, _) in enumerate(slots):
            t_l, t_h = t_l_arr[g], t_h_arr[g]
            kl, kh = t_l * 128, t_h * 128
            a, b = edge_bnds[g], edge_bnds[g + 1]
            sg = s[a:b]
            xg = x[a:b]
            m = xg < SPLIT
            xl = xg[m]
            xh = xg[~m] - SPLIT
            sl_ = sg[m] - base_n
            sh_ = sg[~m] - base_n
            # sort each run by source row: the one-hot matmul is order-
            # invariant within a slot, and address-sorted gather descriptors
            # get HBM row-buffer locality (duplicates become adjacent)
            ol = np.argsort(xl, kind="stable")
            xl, sl_ = xl[ol], sl_[ol]
            oh_ = np.argsort(xh, kind="stable")
            xh, sh_ = xh[oh_], sh_[oh_]
            m0 = int(m_off[g])
            base = m0 * 128
            gidx_all[base : base + len(xl)] = xl.astype(np.int16)
            gidx_all[base + kl : base + kl + len(xh)] = xh.astype(np.int16)
            srl = np.full(kl, -1.0, np.float32)
            srl[: len(sl_)] = sl_
            srel_all[m0 : m0 + t_l] = srl.reshape(t_l, 128)
            srh = np.full(kh, -1.0, np.float32)
            srh[: len(sh_)] = sh_
            srel_all[m0 + t_l : m0 + t_l + t_h] = srh.reshape(t_h, 128)
            abs_base = bounds[c] + base_n
            cnt1[:width, g] = 1.0 + deg[abs_base : abs_base + width]

        gidx_w = np.ascontiguousarray(np.tile(gidx_all.reshape(-1, 16).T, (8, 1)))
        srel_mat = np.ascontiguousarray(srel_all.T)
        in_maps.append(
            {
                "featb": featb,
                "gidx": gidx_w,
                "srel": srel_mat,
                "cnt1": cnt1,
            }
        )
        slot_maps.append(
            [(bounds[c] + sl[0], sl[1]) for sl in slots]
        )
    return t_l_arr, t_h_arr, in_maps, slot_maps


def kernel(features, neighbor_idx, segment_ids):
    global LAST_NC
    t_l_arr, t_h_arr, in_maps, slot_maps = _preprocess(
        features, neighbor_idx, segment_ids
    )

    key = (t_l_arr, t_h_arr)
    if key not in _PROGRAM_CACHE:
        _PROGRAM_CACHE[key] = _build_program(t_l_arr, t_h_arr)
    nc = _PROGRAM_CACHE[key]
    LAST_NC = nc

    try:
        res = bass_utils.run_bass_kernel_spmd(
            nc, in_maps, core_ids=list(range(N_CORES))
        )
    except Exception:
        # transient axon/device hiccups (e.g. recovering from a prior wedge)
        # have been observed to clear after a short pause
        import time

        time.sleep(20)
        res = bass_utils.run_bass_kernel_spmd(
            nc, in_maps, core_ids=list(range(N_CORES))
        )

    out = np.empty((N_NODES, D), np.float32)
    for c in range(N_CORES):
        oc = res.results[c]["out"].astype(np.float32)
        for g, (abs_base, width) in enumerate(slot_maps[c]):
            out[abs_base : abs_base + width] = oc[g * 128 : g * 128 + width]
    return out


# revision 5
# speedup vs baseline: 1.8985x; 1.0599x over previous
"""GCNAggregator Trainium2 Bass kernel.

out[i] = (sum_{e: seg[e]==i} features[neighbor_idx[e]] + features[i]) / (deg_i + 1)

Strategy (8 NeuronCores, SPMD):
  - Nodes are sharded into 8 contiguous, edge-balanced ranges (<=6272 nodes
    each). Since segment_ids is sorted, each core's incident edges are a
    contiguous range of the edge list. The self-loop is folded in as one
    extra edge per node, so the whole aggregation is a single segment-sum.
  - Per core, nodes are packed greedily into "slots" of <=128 consecutive
    nodes, capped by per-slot edge counts so every slot is nearly full of
    edges. Slot tile counts are maxed over the 8 cores so the compiled
    program is identical on every core (SPMD) at ~2% gather padding.
  - Features are stored as a bf16 table (512B/row). Each slot's edges are
    gathered from HBM with gpsimd.dma_gather, then segment-summed on the
    tensor engine via one 256-wide one-hot matmul per 128-edge tile:
        psum[128 nodes, 256] += onehot[128 edges, 128 nodes]^T @ G[128 edges, 256]
    The one-hot is exact in bf16 and PSUM accumulates in fp32 (end-to-end
    rel err ~3e-3, well inside the 2e-2 gate) at 1 cycle/row matmul
    throughput. The one-hot is built on device from per-edge relative
    segment ids (is_equal vs an iota row, all-bf16 for 2x DVE throughput).
  - dma_gather indices are int16 (max 32767) but the table has 50000 rows,
    so each slot's edges are split into a low class (row < 32768) and a
    high class (row >= 32768, gathered from an offset view of the table).
  - The SWDGE descriptor ring is enlarged (dynamic_dma_scratch_size=65536
    -> 4096 descriptors) so each class run is one or two dma_gather calls
    per slot, amortizing the ~1us fixed descriptor-generation cost on the
    Pool engine.
  - Finalize per slot: out = psum * 1/(deg+1) (bf16), DMA out.

The host only computes integer index metadata (shard boundaries, per-slot
class-split index streams, relative segment ids, degrees); all floating
point work (gather, segment sum, normalize) runs on device.
"""

import sys

import numpy as np

try:
    import concourse  # noqa: F401
except ImportError:  # pragma: no cover
    sys.path.insert(0, "/opt/trn_rl_repo")

from contextlib import ExitStack

import concourse.mybir as mybir
from concourse import bacc, bass_utils, tile

N_NODES = 50000
N_EDGES = 1_600_000
D = 256
N_CORES = 8
NPC = 6272          # node slots per core (= GROUPS * 128)
GROUPS = 49
SPLIT = 32768       # int16 gather-index window

_PROGRAM_CACHE: dict = {}
LAST_NC = None  # exposed for test harness introspection (TimelineSim)

import os as _os

DMA_SCRATCH = int(_os.environ.get("DMA_SCRATCH", "16384"))
MAX_GATHER = DMA_SCRATCH // 16  # SWDGE descriptor ring capacity per call


def _chunks(total_rows):
    out = []
    off = 0
    while off < total_rows:
        k = min(MAX_GATHER, total_rows - off)
        out.append((off, k))
        off += k
    return out


def _build_program(t_l_arr: tuple, t_h_arr: tuple):
    """Build + compile the (uniform-across-cores, SPMD) per-core program.

    t_l_arr/t_h_arr: per-group tile counts (max over the 8 cores), so the
    program structure is identical on every core while padding stays low.
    """
    n_slots = len(t_l_arr)
    tiles_g = [t_l_arr[g] + t_h_arr[g] for g in range(n_slots)]
    nt_tot = sum(tiles_g)
    rows_tot = nt_tot * 128
    ni16 = rows_tot // 16  # gidx columns (wrapped-16 int16 layout)
    # column offset of each group's tile block
    m_off = np.concatenate([[0], np.cumsum(tiles_g)]).astype(int)

    nc = bacc.Bacc(
        "TRN2", target_bir_lowering=False, debug=False, num_devices=N_CORES,
        dynamic_dma_scratch_size=DMA_SCRATCH,
    )

    feat_d = nc.dram_tensor(
        "featb", (N_NODES, D), mybir.dt.bfloat16, kind="ExternalInput"
    ).ap()
    gidx_d = nc.dram_tensor(
        "gidx", (128, ni16), mybir.dt.int16, kind="ExternalInput"
    ).ap()
    srel_d = nc.dram_tensor(
        "srel", (128, nt_tot), mybir.dt.float32, kind="ExternalInput"
    ).ap()
    cnt1_d = nc.dram_tensor(
        "cnt1", (128, n_slots), mybir.dt.float32, kind="ExternalInput"
    ).ap()
    out_d = nc.dram_tensor(
        "out", (n_slots * 128, D), mybir.dt.bfloat16, kind="ExternalOutput"
    ).ap()

    feat_lo = feat_d[0:SPLIT, :]
    feat_hi = feat_d[SPLIT:N_NODES, :]

    with tile.TileContext(nc) as tc:
        with ExitStack() as ctx:
            import os

            gb = int(os.environ.get("GT_BUFS", "2"))
            ob = int(os.environ.get("OH_BUFS", "4"))
            fb = int(os.environ.get("FIN_BUFS", "3"))
            pb = int(os.environ.get("PSUM_BUFS", "4"))
            const_pool = ctx.enter_context(tc.tile_pool(name="const", bufs=1))
            g_pool = ctx.enter_context(tc.tile_pool(name="gt", bufs=gb))
            oh_pool = ctx.enter_context(tc.tile_pool(name="oh", bufs=ob))
            fin_pool = ctx.enter_context(tc.tile_pool(name="fin", bufs=fb))
            psum_pool = ctx.enter_context(
                tc.tile_pool(name="psum", bufs=pb, space="PSUM")
            )

            # persistent metadata in SBUF (gidx loaded in chunks so early
            # gathers don't wait on the full index transfer)
            gidx_sb = const_pool.tile([128, ni16], mybir.dt.int16)
            n_ld = 8
            ld_bounds = [ni16 * i // n_ld for i in range(n_ld + 1)]
            for a, b in zip(ld_bounds[:-1], ld_bounds[1:]):
                if b > a:
                    nc.sync.dma_start(gidx_sb[:, a:b], gidx_d[:, a:b])
            srel_sb = const_pool.tile([128, nt_tot], mybir.dt.float32)
            nc.sync.dma_start(srel_sb[:], srel_d[:])
            cnt1_sb = const_pool.tile([128, n_slots], mybir.dt.float32)
            nc.sync.dma_start(cnt1_sb[:], cnt1_d[:])

            iota_i = const_pool.tile([128, 128], mybir.dt.int32)
            nc.gpsimd.iota(iota_i[:], pattern=[[1, 128]], base=0, channel_multiplier=0)
            iota_f = const_pool.tile([128, 128], mybir.dt.bfloat16)
            nc.vector.tensor_copy(iota_f[:], iota_i[:])

            max_tiles = max(tiles_g)
            for g in range(n_slots):
                t_l = t_l_arr[g]
                n_tiles = tiles_g[g]
                m0 = int(m_off[g])
                c0 = m0 * 8  # 128 rows -> 8 int16-wrapped columns
                gt = g_pool.tile(
                    [128, max_tiles, D], mybir.dt.bfloat16, tag="gt"
                )
                for off, k in _chunks(t_l * 128):
                    nc.gpsimd.dma_gather(
                        gt[:, off // 128 : (off + k) // 128, :], feat_lo,
                        gidx_sb[:, c0 + off // 16 : c0 + (off + k) // 16],
                        num_idxs=k, num_idxs_reg=k,
                        elem_size=D, elem_step=D,
                    )
                for off, k in _chunks(t_h_arr[g] * 128):
                    nc.gpsimd.dma_gather(
                        gt[:, t_l + off // 128 : t_l + (off + k) // 128, :],
                        feat_hi,
                        gidx_sb[
                            :,
                            c0 + t_l * 8 + off // 16 : c0 + t_l * 8 + (off + k) // 16,
                        ],
                        num_idxs=k, num_idxs_reg=k,
                        elem_size=D, elem_step=D,
                    )

                psum = psum_pool.tile([128, D], mybir.dt.float32, tag="ps")
                for t in range(n_tiles):
                    oh = oh_pool.tile([128, 128], mybir.dt.bfloat16, tag="oh")
                    nc.vector.tensor_scalar(
                        oh[:], iota_f[:], srel_sb[:, m0 + t : m0 + t + 1], None,
                        op0=mybir.AluOpType.is_equal,
                    )
                    nc.tensor.matmul(
                        psum[:], oh[:], gt[:, t, :],
                        start=(t == 0), stop=(t == n_tiles - 1),
                    )

                rec = fin_pool.tile([128, 1], mybir.dt.float32, tag="rec")
                nc.vector.reciprocal(rec[:], cnt1_sb[:, g : g + 1])
                o_sb = fin_pool.tile([128, D], mybir.dt.bfloat16, tag="o")
                nc.vector.tensor_scalar_mul(o_sb[:], psum[:], rec[:])
                nc.sync.dma_start(out_d[g * 128 : (g + 1) * 128, :], o_sb[:])

    nc.compile()
    return nc


def _pack_slots(cum_l, cum_h, n_nodes, cap_l, cap_h):
    """Greedy variable-width node slots: each slot takes consecutive nodes
    (<=128) while its L/H edge counts stay under the caps. Returns a list of
    (base, width, nL, nH)."""
    slots = []
    i = 0
    while i < n_nodes:
        jmax = min(i + 128, n_nodes)
        jl = int(np.searchsorted(cum_l, cum_l[i] + cap_l * 128, side="right")) - 1
        jh = int(np.searchsorted(cum_h, cum_h[i] + cap_h * 128, side="right")) - 1
        j = max(min(jmax, jl, jh), i + 1)
        slots.append(
            (i, j - i, int(cum_l[j] - cum_l[i]), int(cum_h[j] - cum_h[i]))
        )
        i = j
    return slots


def _preprocess(features, neighbor_idx, segment_ids):
    """Host-side shard/index metadata construction (integers only)."""
    feat = np.ascontiguousarray(np.asarray(features, dtype=np.float32))
    seg = np.asarray(segment_ids).astype(np.int64)
    nid = np.asarray(neighbor_idx).astype(np.int64)
    n_edges = seg.shape[0]

    bf16 = mybir.dt.np(mybir.dt.bfloat16)
    featb = feat.astype(bf16)

    deg = np.bincount(seg, minlength=N_NODES)

    # edge-balanced core node boundaries (spans capped at NPC node slots)
    bounds = [0]
    for c in range(1, N_CORES):
        n = int(seg[min(c * n_edges // N_CORES, n_edges - 1)])
        n = min(n, bounds[-1] + NPC)
        n = max(n, N_NODES - (N_CORES - c) * NPC, bounds[-1])
        bounds.append(n)
    bounds.append(N_NODES)

    # per-core edge slices (self-loop folded in as one extra edge per node)
    # and per-node class-split prefix sums
    per_core = []
    for c in range(N_CORES):
        lo, hi = np.searchsorted(seg, [bounds[c], bounds[c + 1]])
        nn = bounds[c + 1] - bounds[c]
        s = np.concatenate([seg[lo:hi] - bounds[c], np.arange(nn)])
        x = np.concatenate([nid[lo:hi], np.arange(bounds[c], bounds[c + 1])])
        order = np.argsort(s, kind="stable")
        s = s[order]
        x = x[order]
        is_l = x < SPLIT
        cnt_l = np.bincount(s[is_l], minlength=nn)
        cnt_h = np.bincount(s[~is_l], minlength=nn)
        cum_l = np.concatenate([[0], np.cumsum(cnt_l)])
        cum_h = np.concatenate([[0], np.cumsum(cnt_h)])
        per_core.append((s, x, nn, cum_l, cum_h))

    # choose caps minimizing the max of the modeled DMA and Pool-engine
    # (SWDGE descriptor-gen) times: gather descriptors cost ~1.42ns each on
    # the shared DMA engines, while each dma_gather call costs ~1us fixed on
    # the Pool engine with at most MAX_GATHER descriptors per call.
    best = None
    for cap_l in range(8, 27):
        for cap_h in range(4, 15):
            all_slots = [
                _pack_slots(pc[3], pc[4], pc[2], cap_l, cap_h) for pc in per_core
            ]
            n_slots = max(len(sl) for sl in all_slots)
            tl = np.zeros(n_slots, np.int64)
            th = np.zeros(n_slots, np.int64)
            for sl in all_slots:
                for g, (_, _, nl, nh) in enumerate(sl):
                    tl[g] = max(tl[g], -(-nl // 128))
                    th[g] = max(th[g], -(-nh // 128))
            rows = 128 * int(tl.sum() + th.sum())
            calls = sum(
                -(-int(t) * 128 // MAX_GATHER) for t in tl if t
            ) + sum(-(-int(t) * 128 // MAX_GATHER) for t in th if t)
            dma_ns = rows * 1.4225 + 22000
            pool_ns = calls * 994 + rows * 0.34 + 1300
            score = max(dma_ns, pool_ns)
            if best is None or score < best[0]:
                best = (score, tuple(int(v) for v in tl), tuple(int(v) for v in th), all_slots)
    _, t_l_arr, t_h_arr, all_slots = best
    # a slot with zero tiles would leave its PSUM accumulator unwritten
    t_l_arr = tuple(
        max(tl, 1) if tl + th == 0 else tl for tl, th in zip(t_l_arr, t_h_arr)
    )
    n_slots = len(t_l_arr)

    tiles_g = [t_l_arr[g] + t_h_arr[g] for g in range(n_slots)]
    nt_tot = sum(tiles_g)
    m_off = np.concatenate([[0], np.cumsum(tiles_g)]).astype(int)

    in_maps = []
    slot_maps = []
    for c in range(N_CORES):
        s, x, nn, _, _ = per_core[c]
        slots = all_slots[c]
        gidx_all = np.zeros(nt_tot * 128, np.int16)
        srel_all = np.full((nt_tot, 128), -1.0, np.float32)
        cnt1 = np.ones((128, n_slots), np.float32)
        node_bnds = [sl[0] for sl in slots] + [nn]
        edge_bnds = np.searchsorted(s, node_bnds)
        for g, (base_n, width, _, _) in enumerate(slots):
            t_l, t_h = t_l_arr[g], t_h_arr[g]
            kl, kh = t_l * 128, t_h * 128
            a, b = edge_bnds[g], edge_bnds[g + 1]
            sg = s[a:b]
            xg = x[a:b]
            m = xg < SPLIT
            xl = xg[m]
            xh = xg[~m] - SPLIT
            sl_ = sg[m] - base_n
            sh_ = sg[~m] - base_n
            # sort each run by source row: the one-hot matmul is order-
            # invariant within a slot, and address-sorted gather descriptors
            # get HBM row-buffer locality (duplicates become adjacent)
            ol = np.argsort(xl, kind="stable")
            xl, sl_ = xl[ol], sl_[ol]
            oh_ = np.argsort(xh, kind="stable")
            xh, sh_ = xh[oh_], sh_[oh_]
            m0 = int(m_off[g])
            base = m0 * 128
            gidx_all[base : base + len(xl)] = xl.astype(np.int16)
            gidx_all[base + kl : base + kl + len(xh)] = xh.astype(np.int16)
            srl = np.full(kl, -1.0, np.float32)
            srl[: len(sl_)] = sl_
            srel_all[m0 : m0 + t_l] = srl.reshape(t_l, 128)
            srh = np.full(kh, -1.0, np.float32)
            srh[: len(sh_)] = sh_
            srel_all[m0 + t_l : m0 + t_l + t_h] = srh.reshape(t_h, 128)
            abs_base = bounds[c] + base_n
            cnt1[:width, g] = 1.0 + deg[abs_base : abs_base + width]

        gidx_w = np.ascontiguousarray(np.tile(gidx_all.reshape(-1, 16).T, (8, 1)))
        srel_mat = np.ascontiguousarray(srel_all.T)
        in_maps.append(
            {
                "featb": featb,
                "gidx": gidx_w,
                "srel": srel_mat,
                "cnt1": cnt1,
            }
        )
        slot_maps.append(
            [(bounds[c] + sl[0], sl[1]) for sl in slots]
        )
    return t_l_arr, t_h_arr, in_maps, slot_maps


def kernel(features, neighbor_idx, segment_ids):
    global LAST_NC
    t_l_arr, t_h_arr, in_maps, slot_maps = _preprocess(
        features, neighbor_idx, segment_ids
    )

    key = (t_l_arr, t_h_arr)
    if key not in _PROGRAM_CACHE:
        _PROGRAM_CACHE[key] = _build_program(t_l_arr, t_h_arr)
    nc = _PROGRAM_CACHE[key]
    LAST_NC = nc

    try:
        res = bass_utils.run_bass_kernel_spmd(
            nc, in_maps, core_ids=list(range(N_CORES))
        )
    except Exception:
        # transient axon/device hiccups (e.g. recovering from a prior wedge)
        # have been observed to clear after a short pause
        import time

        time.sleep(20)
        res = bass_utils.run_bass_kernel_spmd(
            nc, in_maps, core_ids=list(range(N_CORES))
        )

    out = np.empty((N_NODES, D), np.float32)
    for c in range(N_CORES):
        oc = res.results[c]["out"].astype(np.float32)
        for g, (abs_base, width) in enumerate(slot_maps[c]):
            out[abs_base : abs_base + width] = oc[g * 128 : g * 128 + width]
    return out


# revision 15
# speedup vs baseline: 1.9234x; 1.0131x over previous
"""GCNAggregator Trainium2 Bass kernel.

out[i] = (sum_{e: seg[e]==i} features[neighbor_idx[e]] + features[i]) / (deg_i + 1)

Strategy (8 NeuronCores, SPMD):
  - Nodes are sharded into 8 contiguous, edge-balanced ranges (<=6272 nodes
    each). Since segment_ids is sorted, each core's incident edges are a
    contiguous range of the edge list. The self-loop is folded in as one
    extra edge per node, so the whole aggregation is a single segment-sum.
  - Per core, nodes are packed greedily into "slots" of <=128 consecutive
    nodes, capped by per-slot edge counts so every slot is nearly full of
    edges. Slot tile counts are maxed over the 8 cores so the compiled
    program is identical on every core (SPMD) at ~2% gather padding.
  - Features are stored as a bf16 table (512B/row). Each slot's edges are
    gathered from HBM with gpsimd.dma_gather, then segment-summed on the
    tensor engine via one 256-wide one-hot matmul per 128-edge tile:
        psum[128 nodes, 256] += onehot[128 edges, 128 nodes]^T @ G[128 edges, 256]
    The one-hot is exact in bf16 and PSUM accumulates in fp32 (end-to-end
    rel err ~3e-3, well inside the 2e-2 gate) at 1 cycle/row matmul
    throughput. The one-hot is built on device from per-edge relative
    segment ids (is_equal vs an iota row, all-bf16 for 2x DVE throughput).
  - dma_gather indices are int16 (max 32767) but the table has 50000 rows,
    so each slot's edges are split into a low class (row < 32768) and a
    high class (row >= 32768, gathered from an offset view of the table).
  - The SWDGE descriptor ring is enlarged (dynamic_dma_scratch_size=65536
    -> 4096 descriptors) so each class run is one or two dma_gather calls
    per slot, amortizing the ~1us fixed descriptor-generation cost on the
    Pool engine.
  - Finalize per slot: out = psum * 1/(deg+1) (bf16), DMA out.

The host only computes integer index metadata (shard boundaries, per-slot
class-split index streams, relative segment ids, degrees); all floating
point work (gather, segment sum, normalize) runs on device.
"""

import sys

import numpy as np

try:
    import concourse  # noqa: F401
except ImportError:  # pragma: no cover
    sys.path.insert(0, "/opt/trn_rl_repo")

from contextlib import ExitStack

import concourse.mybir as mybir
from concourse import bacc, bass_utils, tile

N_NODES = 50000
N_EDGES = 1_600_000
D = 256
N_CORES = 8
NPC = 6272          # node slots per core (= GROUPS * 128)
GROUPS = 49
SPLIT = 32768       # int16 gather-index window

_PROGRAM_CACHE: dict = {}
LAST_NC = None  # exposed for test harness introspection (TimelineSim)

import os as _os

DMA_SCRATCH = int(_os.environ.get("DMA_SCRATCH", "16384"))
MAX_GATHER = DMA_SCRATCH // 16  # SWDGE descriptor ring capacity per call
GIDX16 = _os.environ.get("GIDX16", "0") == "1"  # un-replicated 16-row gidx
GIDX_P = 16 if GIDX16 else 128
# replicate the 16-row wrapped gather-index stream to 128 partitions on the
# tensor engine (f32 one-hot matmul, exact for idx values < 2^24) instead of
# shipping it 8x-replicated over the wire
GIDXPE = _os.environ.get("GIDXPE", "0") == "1"


RING_L = 96   # L-stream SBUF ring, in 128-row tiles (multiple of 8)
RING_H = 56   # H-stream ring


def _build_program(t_l_arr: tuple, t_h_arr: tuple):
    """Build + compile the (uniform-across-cores, SPMD) per-core program.

    t_l_arr/t_h_arr: per-group tile counts (max over the 8 cores), so the
    program structure is identical on every core while padding stays low.

    The L- and H-class gather rows of all slots form two contiguous streams;
    each stream is gathered with full MAX_GATHER-descriptor dma_gather calls
    (independent of slot boundaries) into a circular SBUF ring of 128-row
    tiles. Slot matmuls read their tiles from the rings; the tile
    framework's subtile dependency tracking orders ring reuse.
    """
    n_slots = len(t_l_arr)
    tiles_g = [t_l_arr[g] + t_h_arr[g] for g in range(n_slots)]
    nt_tot = sum(tiles_g)
    rows_tot = nt_tot * 128
    ni16 = rows_tot // 16  # gidx columns (wrapped-16 int16 layout)
    nt_l = sum(t_l_arr)
    nt_h = sum(t_h_arr)
    rows_l = nt_l * 128
    cum_lt = np.concatenate([[0], np.cumsum(t_l_arr)]).astype(int)
    cum_ht = np.concatenate([[0], np.cumsum(t_h_arr)]).astype(int)

    nc = bacc.Bacc(
        "TRN2", target_bir_lowering=False, debug=False, num_devices=N_CORES,
        dynamic_dma_scratch_size=DMA_SCRATCH,
    )

    feat_d = nc.dram_tensor(
        "featb", (N_NODES, D), mybir.dt.bfloat16, kind="ExternalInput"
    ).ap()
    if GIDXPE:
        gidx_d = nc.dram_tensor(
            "gidxf", (16, ni16), mybir.dt.float32, kind="ExternalInput"
        ).ap()
        rep_d = nc.dram_tensor(
            "repmat", (16, 128), mybir.dt.float32, kind="ExternalInput"
        ).ap()
    else:
        gidx_d = nc.dram_tensor(
            "gidx", (GIDX_P, ni16), mybir.dt.int16, kind="ExternalInput"
        ).ap()
    srel_d = nc.dram_tensor(
        "srel", (128, nt_tot), mybir.dt.bfloat16, kind="ExternalInput"
    ).ap()
    cnt1_d = nc.dram_tensor(
        "cnt1", (128, n_slots), mybir.dt.float32, kind="ExternalInput"
    ).ap()
    out_d = nc.dram_tensor(
        "out", (n_slots * 128, D), mybir.dt.bfloat16, kind="ExternalOutput"
    ).ap()

    feat_lo = feat_d[0:SPLIT, :]
    feat_hi = feat_d[SPLIT:N_NODES, :]

    with tile.TileContext(nc) as tc:
        with ExitStack() as ctx:
            import os

            ob = int(os.environ.get("OH_BUFS", "4"))
            fb = int(os.environ.get("FIN_BUFS", "3"))
            pb = int(os.environ.get("PSUM_BUFS", "4"))
            const_pool = ctx.enter_context(tc.tile_pool(name="const", bufs=1))
            oh_pool = ctx.enter_context(tc.tile_pool(name="oh", bufs=ob))
            fin_pool = ctx.enter_context(tc.tile_pool(name="fin", bufs=fb))
            psum_pool = ctx.enter_context(
                tc.tile_pool(name="psum", bufs=pb, space="PSUM")
            )

            # persistent metadata in SBUF (gidx loaded in chunks so early
            # gathers don't wait on the full index transfer)
            if GIDXPE:
                gidx_sb = const_pool.tile([128, ni16], mybir.dt.int16)
                gidxf_sb = const_pool.tile([16, ni16], mybir.dt.float32)
                rep_sb = const_pool.tile([16, 128], mybir.dt.float32)
                nc.sync.dma_start(rep_sb[:], rep_d[:])
                n_ld = 8
                ld_bounds = [ni16 * i // n_ld for i in range(n_ld + 1)]
                for a, b in zip(ld_bounds[:-1], ld_bounds[1:]):
                    if b > a:
                        nc.sync.dma_start(gidxf_sb[:, a:b], gidx_d[:, a:b])
                rep_pool = ctx.enter_context(
                    tc.tile_pool(name="rpsum", bufs=2, space="PSUM")
                )
                for a in range(0, ni16, 512):
                    b = min(a + 512, ni16)
                    rp = rep_pool.tile([128, 512], mybir.dt.float32, tag="rp")
                    nc.tensor.matmul(
                        rp[:, : b - a], rep_sb[:], gidxf_sb[:, a:b],
                        start=True, stop=True,
                    )
                    nc.vector.tensor_copy(gidx_sb[:, a:b], rp[:, : b - a])
            else:
                gidx_sb = const_pool.tile([GIDX_P, ni16], mybir.dt.int16)
                n_ld = 8
                ld_bounds = [ni16 * i // n_ld for i in range(n_ld + 1)]
                for a, b in zip(ld_bounds[:-1], ld_bounds[1:]):
                    if b > a:
                        nc.sync.dma_start(gidx_sb[:, a:b], gidx_d[:, a:b])
            # srel rides the wire as bf16 (values are small integers, exact)
            # and is widened on device: tensor_scalar's scalar operand must
            # be f32.
            srel_bf = const_pool.tile([128, nt_tot], mybir.dt.bfloat16)
            nc.sync.dma_start(srel_bf[:], srel_d[:])
            srel_sb = const_pool.tile([128, nt_tot], mybir.dt.float32)
            nc.vector.tensor_copy(srel_sb[:], srel_bf[:])
            cnt1_sb = const_pool.tile([128, n_slots], mybir.dt.float32)
            nc.sync.dma_start(cnt1_sb[:], cnt1_d[:])

            iota_i = const_pool.tile([128, 128], mybir.dt.int32)
            nc.gpsimd.iota(iota_i[:], pattern=[[1, 128]], base=0, channel_multiplier=0)
            iota_f = const_pool.tile([128, 128], mybir.dt.bfloat16)
            nc.vector.tensor_copy(iota_f[:], iota_i[:])

            ring_l = const_pool.tile([128, RING_L, D], mybir.dt.bfloat16)
            ring_h = const_pool.tile([128, RING_H, D], mybir.dt.bfloat16)

            def emit_call(ring, ring_sz, src, row0, rows_end, col0):
                """One full-ring-slice gather call of the given stream."""
                k = min(MAX_GATHER, rows_end - row0)
                s0 = (row0 // 128) % ring_sz
                nc.gpsimd.dma_gather(
                    ring[:, s0 : s0 + k // 128, :], src,
                    gidx_sb[:, col0 + row0 // 16 : col0 + (row0 + k) // 16],
                    num_idxs=k, num_idxs_reg=k,
                    elem_size=D, elem_step=D,
                )
                return row0 + k

            done_l = 0  # stream rows gathered so far
            done_h = 0
            for g in range(n_slots):
                t_l = t_l_arr[g]
                n_tiles = tiles_g[g]
                while done_l < cum_lt[g + 1] * 128:
                    done_l = emit_call(ring_l, RING_L, feat_lo, done_l,
                                       nt_l * 128, 0)
                while done_h < cum_ht[g + 1] * 128:
                    done_h = emit_call(ring_h, RING_H, feat_hi, done_h,
                                       nt_h * 128, rows_l // 16)

                psum = psum_pool.tile([128, D], mybir.dt.float32, tag="ps")
                for t in range(n_tiles):
                    if t < t_l:
                        m = int(cum_lt[g]) + t
                        gt_tile = ring_l[:, m % RING_L, :]
                    else:
                        m = nt_l + int(cum_ht[g]) + (t - t_l)
                        gt_tile = ring_h[:, (m - nt_l) % RING_H, :]
                    oh = oh_pool.tile([128, 128], mybir.dt.bfloat16, tag="oh")
                    nc.vector.tensor_scalar(
                        oh[:], iota_f[:], srel_sb[:, m : m + 1], None,
                        op0=mybir.AluOpType.is_equal,
                    )
                    nc.tensor.matmul(
                        psum[:], oh[:], gt_tile,
                        start=(t == 0), stop=(t == n_tiles - 1),
                    )

                rec = fin_pool.tile([128, 1], mybir.dt.float32, tag="rec")
                nc.vector.reciprocal(rec[:], cnt1_sb[:, g : g + 1])
                o_sb = fin_pool.tile([128, D], mybir.dt.bfloat16, tag="o")
                nc.vector.tensor_scalar_mul(o_sb[:], psum[:], rec[:])
                nc.sync.dma_start(out_d[g * 128 : (g + 1) * 128, :], o_sb[:])

    nc.compile()
    return nc


def _pack_slots(cum_l, cum_h, n_nodes, cap_l, cap_h):
    """Greedy variable-width node slots: each slot takes consecutive nodes
    (<=128) while its L/H edge counts stay under the caps. Returns a list of
    (base, width, nL, nH)."""
    slots = []
    i = 0
    while i < n_nodes:
        jmax = min(i + 128, n_nodes)
        jl = int(np.searchsorted(cum_l, cum_l[i] + cap_l * 128, side="right")) - 1
        jh = int(np.searchsorted(cum_h, cum_h[i] + cap_h * 128, side="right")) - 1
        j = max(min(jmax, jl, jh), i + 1)
        slots.append(
            (i, j - i, int(cum_l[j] - cum_l[i]), int(cum_h[j] - cum_h[i]))
        )
        i = j
    return slots


def _preprocess(features, neighbor_idx, segment_ids):
    """Host-side shard/index metadata construction (integers only)."""
    feat = np.ascontiguousarray(np.asarray(features, dtype=np.float32))
    seg = np.asarray(segment_ids).astype(np.int64)
    nid = np.asarray(neighbor_idx).astype(np.int64)
    n_edges = seg.shape[0]

    bf16 = mybir.dt.np(mybir.dt.bfloat16)
    featb = feat.astype(bf16)

    deg = np.bincount(seg, minlength=N_NODES)

    # edge-balanced core node boundaries (spans capped at NPC node slots)
    bounds = [0]
    for c in range(1, N_CORES):
        n = int(seg[min(c * n_edges // N_CORES, n_edges - 1)])
        n = min(n, bounds[-1] + NPC)
        n = max(n, N_NODES - (N_CORES - c) * NPC, bounds[-1])
        bounds.append(n)
    bounds.append(N_NODES)

    # per-core edge slices (self-loop folded in as one extra edge per node)
    # and per-node class-split prefix sums
    per_core = []
    for c in range(N_CORES):
        lo, hi = np.searchsorted(seg, [bounds[c], bounds[c + 1]])
        nn = bounds[c + 1] - bounds[c]
        s = np.concatenate([seg[lo:hi] - bounds[c], np.arange(nn)])
        x = np.concatenate([nid[lo:hi], np.arange(bounds[c], bounds[c + 1])])
        order = np.argsort(s, kind="stable")
        s = s[order]
        x = x[order]
        is_l = x < SPLIT
        cnt_l = np.bincount(s[is_l], minlength=nn)
        cnt_h = np.bincount(s[~is_l], minlength=nn)
        cum_l = np.concatenate([[0], np.cumsum(cnt_l)])
        cum_h = np.concatenate([[0], np.cumsum(cnt_h)])
        per_core.append((s, x, nn, cum_l, cum_h))

    # choose caps minimizing the max of the modeled DMA and Pool-engine
    # (SWDGE descriptor-gen) times: gather descriptors cost ~1.42ns each on
    # the shared DMA engines, while each dma_gather call costs ~1us fixed on
    # the Pool engine with at most MAX_GATHER descriptors per call.
    best = None
    for cap_l in range(8, 27):
        for cap_h in range(4, 15):
            all_slots = [
                _pack_slots(pc[3], pc[4], pc[2], cap_l, cap_h) for pc in per_core
            ]
            n_slots = max(len(sl) for sl in all_slots)
            tl = np.zeros(n_slots, np.int64)
            th = np.zeros(n_slots, np.int64)
            for sl in all_slots:
                for g, (_, _, nl, nh) in enumerate(sl):
                    tl[g] = max(tl[g], -(-nl // 128))
                    th[g] = max(th[g], -(-nh // 128))
            rows = 128 * int(tl.sum() + th.sum())
            calls = -(-128 * int(tl.sum()) // MAX_GATHER) + -(
                -128 * int(th.sum()) // MAX_GATHER
            )
            dma_ns = rows * 1.4225 + (13000 if GIDX16 else 22000)
            pool_ns = calls * 994 + rows * 0.34 + 1300
            score = max(dma_ns, pool_ns)
            if best is None or score < best[0]:
                best = (score, tuple(int(v) for v in tl), tuple(int(v) for v in th), all_slots)
    _, t_l_arr, t_h_arr, all_slots = best
    # a slot with zero tiles would leave its PSUM accumulator unwritten
    t_l_arr = tuple(
        max(tl, 1) if tl + th == 0 else tl for tl, th in zip(t_l_arr, t_h_arr)
    )
    n_slots = len(t_l_arr)

    nt_tot = sum(t_l_arr) + sum(t_h_arr)
    nt_l = sum(t_l_arr)
    rows_l = nt_l * 128
    cum_lt = np.concatenate([[0], np.cumsum(t_l_arr)]).astype(int)
    cum_ht = np.concatenate([[0], np.cumsum(t_h_arr)]).astype(int)

    in_maps = []
    slot_maps = []
    for c in range(N_CORES):
        s, x, nn, _, _ = per_core[c]
        slots = all_slots[c]
        gidx_all = np.zeros(nt_tot * 128, np.int16)
        srel_all = np.full((nt_tot, 128), -1.0, np.float32)
        cnt1 = np.ones((128, n_slots), np.float32)
        node_bnds = [sl[0] for sl in slots] + [nn]
        edge_bnds = np.searchsorted(s, node_bnds)
        for g, (base_n, width, _, _) in enumerate(slots):
            t_l, t_h = t_l_arr[g], t_h_arr[g]
            kl, kh = t_l * 128, t_h * 128
            a, b = edge_bnds[g], edge_bnds[g + 1]
            sg = s[a:b]
            xg = x[a:b]
            m = xg < SPLIT
            xl = xg[m]
            xh = xg[~m] - SPLIT
            sl_ = sg[m] - base_n
            sh_ = sg[~m] - base_n
            # sort each run by source row: the one-hot matmul is order-
            # invariant within a slot, and address-sorted gather descriptors
            # get HBM row-buffer locality (duplicates become adjacent)
            ol = np.argsort(xl, kind="stable")
            xl, sl_ = xl[ol], sl_[ol]
            oh_ = np.argsort(xh, kind="stable")
            xh, sh_ = xh[oh_], sh_[oh_]
            base_l = int(cum_lt[g]) * 128
            base_h = rows_l + int(cum_ht[g]) * 128
            gidx_all[base_l : base_l + len(xl)] = xl.astype(np.int16)
            gidx_all[base_h : base_h + len(xh)] = xh.astype(np.int16)
            srl = np.full(kl, -1.0, np.float32)
            srl[: len(sl_)] = sl_
            srel_all[cum_lt[g] : cum_lt[g] + t_l] = srl.reshape(t_l, 128)
            srh = np.full(kh, -1.0, np.float32)
            srh[: len(sh_)] = sh_
            srel_all[nt_l + cum_ht[g] : nt_l + cum_ht[g] + t_h] = (
                srh.reshape(t_h, 128)
            )
            abs_base = bounds[c] + base_n
            cnt1[:width, g] = 1.0 + deg[abs_base : abs_base + width]

        gidx_w = gidx_all.reshape(-1, 16).T
        if GIDXPE:
            gidx_w = np.ascontiguousarray(gidx_w.astype(np.float32))
        else:
            if not GIDX16:
                gidx_w = np.tile(gidx_w, (8, 1))
            gidx_w = np.ascontiguousarray(gidx_w)
        srel_mat = np.ascontiguousarray(srel_all.T).astype(bf16)
        imap = {
            "featb": featb,
            "srel": srel_mat,
            "cnt1": cnt1,
        }
        if GIDXPE:
            imap["gidxf"] = gidx_w
            imap["repmat"] = np.ascontiguousarray(
                (np.arange(128)[None, :] % 16 == np.arange(16)[:, None])
                .astype(np.float32)
            )
        else:
            imap["gidx"] = gidx_w
        in_maps.append(imap)
        slot_maps.append(
            [(bounds[c] + sl[0], sl[1]) for sl in slots]
        )
    return t_l_arr, t_h_arr, in_maps, slot_maps


def kernel(features, neighbor_idx, segment_ids):
    global LAST_NC
    t_l_arr, t_h_arr, in_maps, slot_maps = _preprocess(
        features, neighbor_idx, segment_ids
    )

    key = (t_l_arr, t_h_arr)
    if key not in _PROGRAM_CACHE:
        _PROGRAM_CACHE[key] = _build_program(t_l_arr, t_h_arr)
    nc = _PROGRAM_CACHE[key]
    LAST_NC = nc

    try:
        res = bass_utils.run_bass_kernel_spmd(
            nc, in_maps, core_ids=list(range(N_CORES))
        )
    except Exception:
        # transient axon/device hiccups (e.g. recovering from a prior wedge)
        # have been observed to clear after a short pause
        import time

        time.sleep(20)
        res = bass_utils.run_bass_kernel_spmd(
            nc, in_maps, core_ids=list(range(N_CORES))
        )

    out = np.empty((N_NODES, D), np.float32)
    for c in range(N_CORES):
        oc = res.results[c]["out"].astype(np.float32)
        for g, (abs_base, width) in enumerate(slot_maps[c]):
            out[abs_base : abs_base + width] = oc[g * 128 : g * 128 + width]
    return out


# revision 16
# speedup vs baseline: 1.9303x; 1.0036x over previous
"""GCNAggregator Trainium2 Bass kernel.

out[i] = (sum_{e: seg[e]==i} features[neighbor_idx[e]] + features[i]) / (deg_i + 1)

Strategy (8 NeuronCores, SPMD):
  - Nodes are sharded into 8 contiguous, edge-balanced ranges (<=6272 nodes
    each). Since segment_ids is sorted, each core's incident edges are a
    contiguous range of the edge list. The self-loop is folded in as one
    extra edge per node, so the whole aggregation is a single segment-sum.
  - Per core, nodes are packed greedily into "slots" of <=128 consecutive
    nodes, capped by per-slot edge counts so every slot is nearly full of
    edges. Slot tile counts are maxed over the 8 cores so the compiled
    program is identical on every core (SPMD) at ~2% gather padding.
  - Features are stored as a bf16 table (512B/row). Each slot's edges are
    gathered from HBM with gpsimd.dma_gather, then segment-summed on the
    tensor engine via one 256-wide one-hot matmul per 128-edge tile:
        psum[128 nodes, 256] += onehot[128 edges, 128 nodes]^T @ G[128 edges, 256]
    The one-hot is exact in bf16 and PSUM accumulates in fp32 (end-to-end
    rel err ~3e-3, well inside the 2e-2 gate) at 1 cycle/row matmul
    throughput. The one-hot is built on device from per-edge relative
    segment ids (is_equal vs an iota row, all-bf16 for 2x DVE throughput).
  - dma_gather indices are int16 (max 32767) but the table has 50000 rows,
    so each slot's edges are split into a low class (row < 32768) and a
    high class (row >= 32768, gathered from an offset view of the table).
  - The SWDGE descriptor ring is enlarged (dynamic_dma_scratch_size=65536
    -> 4096 descriptors) so each class run is one or two dma_gather calls
    per slot, amortizing the ~1us fixed descriptor-generation cost on the
    Pool engine.
  - Finalize per slot: out = psum * 1/(deg+1) (bf16), DMA out.

The host only computes integer index metadata (shard boundaries, per-slot
class-split index streams, relative segment ids, degrees); all floating
point work (gather, segment sum, normalize) runs on device.
"""

import sys

import numpy as np

try:
    import concourse  # noqa: F401
except ImportError:  # pragma: no cover
    sys.path.insert(0, "/opt/trn_rl_repo")

from contextlib import ExitStack

import concourse.mybir as mybir
from concourse import bacc, bass_utils, tile

N_NODES = 50000
N_EDGES = 1_600_000
D = 256
N_CORES = 8
NPC = 6272          # node slots per core (= GROUPS * 128)
GROUPS = 49
SPLIT = 32768       # int16 gather-index window

_PROGRAM_CACHE: dict = {}
LAST_NC = None  # exposed for test harness introspection (TimelineSim)

import os as _os

DMA_SCRATCH = int(_os.environ.get("DMA_SCRATCH", "16384"))
MAX_GATHER = DMA_SCRATCH // 16  # SWDGE descriptor ring capacity per call
GIDX16 = _os.environ.get("GIDX16", "0") == "1"  # un-replicated 16-row gidx
GIDX_P = 16 if GIDX16 else 128
# replicate the 16-row wrapped gather-index stream to 128 partitions on the
# tensor engine (f32 one-hot matmul, exact for idx values < 2^24) instead of
# shipping it 8x-replicated over the wire
GIDXPE = _os.environ.get("GIDXPE", "0") == "1"


RING_L = 96   # L-stream SBUF ring, in 128-row tiles (multiple of 8)
RING_H = 56   # H-stream ring


def _build_program(t_l_arr: tuple, t_h_arr: tuple):
    """Build + compile the (uniform-across-cores, SPMD) per-core program.

    t_l_arr/t_h_arr: per-group tile counts (max over the 8 cores), so the
    program structure is identical on every core while padding stays low.

    The L- and H-class gather rows of all slots form two contiguous streams;
    each stream is gathered with full MAX_GATHER-descriptor dma_gather calls
    (independent of slot boundaries) into a circular SBUF ring of 128-row
    tiles. Slot matmuls read their tiles from the rings; the tile
    framework's subtile dependency tracking orders ring reuse.
    """
    n_slots = len(t_l_arr)
    tiles_g = [t_l_arr[g] + t_h_arr[g] for g in range(n_slots)]
    nt_tot = sum(tiles_g)
    rows_tot = nt_tot * 128
    ni16 = rows_tot // 16  # gidx columns (wrapped-16 int16 layout)
    nt_l = sum(t_l_arr)
    nt_h = sum(t_h_arr)
    rows_l = nt_l * 128
    cum_lt = np.concatenate([[0], np.cumsum(t_l_arr)]).astype(int)
    cum_ht = np.concatenate([[0], np.cumsum(t_h_arr)]).astype(int)

    nc = bacc.Bacc(
        "TRN2", target_bir_lowering=False, debug=False, num_devices=N_CORES,
        dynamic_dma_scratch_size=DMA_SCRATCH,
    )

    feat_d = nc.dram_tensor(
        "featb", (N_NODES, D), mybir.dt.bfloat16, kind="ExternalInput"
    ).ap()
    if GIDXPE:
        gidx_d = nc.dram_tensor(
            "gidxf", (16, ni16), mybir.dt.float32, kind="ExternalInput"
        ).ap()
        rep_d = nc.dram_tensor(
            "repmat", (16, 128), mybir.dt.float32, kind="ExternalInput"
        ).ap()
    else:
        gidx_d = nc.dram_tensor(
            "gidx", (GIDX_P, ni16), mybir.dt.int16, kind="ExternalInput"
        ).ap()
    srel_d = nc.dram_tensor(
        "srel", (128, nt_tot), mybir.dt.bfloat16, kind="ExternalInput"
    ).ap()
    cnt1_d = nc.dram_tensor(
        "cnt1", (128, n_slots), mybir.dt.float32, kind="ExternalInput"
    ).ap()
    out_d = nc.dram_tensor(
        "out", (n_slots * 128, D), mybir.dt.bfloat16, kind="ExternalOutput"
    ).ap()

    feat_lo = feat_d[0:SPLIT, :]
    feat_hi = feat_d[SPLIT:N_NODES, :]

    with tile.TileContext(nc) as tc:
        with ExitStack() as ctx:
            import os

            ob = int(os.environ.get("OH_BUFS", "4"))
            fb = int(os.environ.get("FIN_BUFS", "3"))
            pb = int(os.environ.get("PSUM_BUFS", "4"))
            const_pool = ctx.enter_context(tc.tile_pool(name="const", bufs=1))
            oh_pool = ctx.enter_context(tc.tile_pool(name="oh", bufs=ob))
            fin_pool = ctx.enter_context(tc.tile_pool(name="fin", bufs=fb))
            psum_pool = ctx.enter_context(
                tc.tile_pool(name="psum", bufs=pb, space="PSUM")
            )

            # persistent metadata in SBUF (gidx loaded in chunks so early
            # gathers don't wait on the full index transfer)
            if GIDXPE:
                gidx_sb = const_pool.tile([128, ni16], mybir.dt.int16)
                gidxf_sb = const_pool.tile([16, ni16], mybir.dt.float32)
                rep_sb = const_pool.tile([16, 128], mybir.dt.float32)
                nc.sync.dma_start(rep_sb[:], rep_d[:])
                n_ld = 8
                ld_bounds = [ni16 * i // n_ld for i in range(n_ld + 1)]
                for a, b in zip(ld_bounds[:-1], ld_bounds[1:]):
                    if b > a:
                        nc.sync.dma_start(gidxf_sb[:, a:b], gidx_d[:, a:b])
                rep_pool = ctx.enter_context(
                    tc.tile_pool(name="rpsum", bufs=2, space="PSUM")
                )
                rep_bounds = [0]
                while rep_bounds[-1] < ni16:
                    a = rep_bounds[-1]
                    step = 128 if a < 1024 else 512
                    rep_bounds.append(min(a + step, ni16))
                for a, b in zip(rep_bounds[:-1], rep_bounds[1:]):
                    rp = rep_pool.tile([128, 512], mybir.dt.float32, tag="rp")
                    nc.tensor.matmul(
                        rp[:, : b - a], rep_sb[:], gidxf_sb[:, a:b],
                        start=True, stop=True,
                    )
                    nc.vector.tensor_copy(gidx_sb[:, a:b], rp[:, : b - a])
            else:
                gidx_sb = const_pool.tile([GIDX_P, ni16], mybir.dt.int16)
                n_ld = 8
                ld_bounds = [ni16 * i // n_ld for i in range(n_ld + 1)]
                for a, b in zip(ld_bounds[:-1], ld_bounds[1:]):
                    if b > a:
                        nc.sync.dma_start(gidx_sb[:, a:b], gidx_d[:, a:b])
            # srel rides the wire as bf16 (values are small integers, exact)
            # and is widened on device: tensor_scalar's scalar operand must
            # be f32.
            srel_bf = const_pool.tile([128, nt_tot], mybir.dt.bfloat16)
            nc.sync.dma_start(srel_bf[:], srel_d[:])
            srel_sb = const_pool.tile([128, nt_tot], mybir.dt.float32)
            nc.vector.tensor_copy(srel_sb[:], srel_bf[:])
            cnt1_sb = const_pool.tile([128, n_slots], mybir.dt.float32)
            nc.sync.dma_start(cnt1_sb[:], cnt1_d[:])

            iota_i = const_pool.tile([128, 128], mybir.dt.int32)
            nc.gpsimd.iota(iota_i[:], pattern=[[1, 128]], base=0, channel_multiplier=0)
            iota_f = const_pool.tile([128, 128], mybir.dt.bfloat16)
            nc.vector.tensor_copy(iota_f[:], iota_i[:])

            ring_l = const_pool.tile([128, RING_L, D], mybir.dt.bfloat16)
            ring_h = const_pool.tile([128, RING_H, D], mybir.dt.bfloat16)

            def emit_call(ring, ring_sz, src, row0, rows_end, col0):
                """One full-ring-slice gather call of the given stream."""
                k = min(MAX_GATHER, rows_end - row0)
                s0 = (row0 // 128) % ring_sz
                nc.gpsimd.dma_gather(
                    ring[:, s0 : s0 + k // 128, :], src,
                    gidx_sb[:, col0 + row0 // 16 : col0 + (row0 + k) // 16],
                    num_idxs=k, num_idxs_reg=k,
                    elem_size=D, elem_step=D,
                )
                return row0 + k

            done_l = 0  # stream rows gathered so far
            done_h = 0
            for g in range(n_slots):
                t_l = t_l_arr[g]
                n_tiles = tiles_g[g]
                while done_l < cum_lt[g + 1] * 128:
                    done_l = emit_call(ring_l, RING_L, feat_lo, done_l,
                                       nt_l * 128, 0)
                while done_h < cum_ht[g + 1] * 128:
                    done_h = emit_call(ring_h, RING_H, feat_hi, done_h,
                                       nt_h * 128, rows_l // 16)

                psum = psum_pool.tile([128, D], mybir.dt.float32, tag="ps")
                for t in range(n_tiles):
                    if t < t_l:
                        m = int(cum_lt[g]) + t
                        gt_tile = ring_l[:, m % RING_L, :]
                    else:
                        m = nt_l + int(cum_ht[g]) + (t - t_l)
                        gt_tile = ring_h[:, (m - nt_l) % RING_H, :]
                    oh = oh_pool.tile([128, 128], mybir.dt.bfloat16, tag="oh")
                    nc.vector.tensor_scalar(
                        oh[:], iota_f[:], srel_sb[:, m : m + 1], None,
                        op0=mybir.AluOpType.is_equal,
                    )
                    nc.tensor.matmul(
                        psum[:], oh[:], gt_tile,
                        start=(t == 0), stop=(t == n_tiles - 1),
                    )

                rec = fin_pool.tile([128, 1], mybir.dt.float32, tag="rec")
                nc.vector.reciprocal(rec[:], cnt1_sb[:, g : g + 1])
                o_sb = fin_pool.tile([128, D], mybir.dt.bfloat16, tag="o")
                nc.vector.tensor_scalar_mul(o_sb[:], psum[:], rec[:])
                nc.sync.dma_start(out_d[g * 128 : (g + 1) * 128, :], o_sb[:])

    nc.compile()
    return nc


def _pack_slots(cum_l, cum_h, n_nodes, cap_l, cap_h):
    """Greedy variable-width node slots: each slot takes consecutive nodes
    (<=128) while its L/H edge counts stay under the caps. Returns a list of
    (base, width, nL, nH)."""
    slots = []
    i = 0
    while i < n_nodes:
        jmax = min(i + 128, n_nodes)
        jl = int(np.searchsorted(cum_l, cum_l[i] + cap_l * 128, side="right")) - 1
        jh = int(np.searchsorted(cum_h, cum_h[i] + cap_h * 128, side="right")) - 1
        j = max(min(jmax, jl, jh), i + 1)
        slots.append(
            (i, j - i, int(cum_l[j] - cum_l[i]), int(cum_h[j] - cum_h[i]))
        )
        i = j
    return slots


def _preprocess(features, neighbor_idx, segment_ids):
    """Host-side shard/index metadata construction (integers only)."""
    feat = np.ascontiguousarray(np.asarray(features, dtype=np.float32))
    seg = np.asarray(segment_ids).astype(np.int64)
    nid = np.asarray(neighbor_idx).astype(np.int64)
    n_edges = seg.shape[0]

    bf16 = mybir.dt.np(mybir.dt.bfloat16)
    featb = feat.astype(bf16)

    deg = np.bincount(seg, minlength=N_NODES)

    # edge-balanced core node boundaries (spans capped at NPC node slots)
    bounds = [0]
    for c in range(1, N_CORES):
        n = int(seg[min(c * n_edges // N_CORES, n_edges - 1)])
        n = min(n, bounds[-1] + NPC)
        n = max(n, N_NODES - (N_CORES - c) * NPC, bounds[-1])
        bounds.append(n)
    bounds.append(N_NODES)

    # per-core edge slices (self-loop folded in as one extra edge per node)
    # and per-node class-split prefix sums
    per_core = []
    for c in range(N_CORES):
        lo, hi = np.searchsorted(seg, [bounds[c], bounds[c + 1]])
        nn = bounds[c + 1] - bounds[c]
        s = np.concatenate([seg[lo:hi] - bounds[c], np.arange(nn)])
        x = np.concatenate([nid[lo:hi], np.arange(bounds[c], bounds[c + 1])])
        order = np.argsort(s, kind="stable")
        s = s[order]
        x = x[order]
        is_l = x < SPLIT
        cnt_l = np.bincount(s[is_l], minlength=nn)
        cnt_h = np.bincount(s[~is_l], minlength=nn)
        cum_l = np.concatenate([[0], np.cumsum(cnt_l)])
        cum_h = np.concatenate([[0], np.cumsum(cnt_h)])
        per_core.append((s, x, nn, cum_l, cum_h))

    # choose caps minimizing the max of the modeled DMA and Pool-engine
    # (SWDGE descriptor-gen) times: gather descriptors cost ~1.42ns each on
    # the shared DMA engines, while each dma_gather call costs ~1us fixed on
    # the Pool engine with at most MAX_GATHER descriptors per call.
    best = None
    for cap_l in range(8, 27):
        for cap_h in range(4, 15):
            all_slots = [
                _pack_slots(pc[3], pc[4], pc[2], cap_l, cap_h) for pc in per_core
            ]
            n_slots = max(len(sl) for sl in all_slots)
            tl = np.zeros(n_slots, np.int64)
            th = np.zeros(n_slots, np.int64)
            for sl in all_slots:
                for g, (_, _, nl, nh) in enumerate(sl):
                    tl[g] = max(tl[g], -(-nl // 128))
                    th[g] = max(th[g], -(-nh // 128))
            rows = 128 * int(tl.sum() + th.sum())
            calls = -(-128 * int(tl.sum()) // MAX_GATHER) + -(
                -128 * int(th.sum()) // MAX_GATHER
            )
            dma_ns = rows * 1.4225 + (13000 if GIDX16 else 22000)
            pool_ns = calls * 994 + rows * 0.34 + 1300
            score = max(dma_ns, pool_ns)
            if best is None or score < best[0]:
                best = (score, tuple(int(v) for v in tl), tuple(int(v) for v in th), all_slots)
    _, t_l_arr, t_h_arr, all_slots = best
    # a slot with zero tiles would leave its PSUM accumulator unwritten
    t_l_arr = tuple(
        max(tl, 1) if tl + th == 0 else tl for tl, th in zip(t_l_arr, t_h_arr)
    )
    n_slots = len(t_l_arr)

    nt_tot = sum(t_l_arr) + sum(t_h_arr)
    nt_l = sum(t_l_arr)
    rows_l = nt_l * 128
    cum_lt = np.concatenate([[0], np.cumsum(t_l_arr)]).astype(int)
    cum_ht = np.concatenate([[0], np.cumsum(t_h_arr)]).astype(int)

    in_maps = []
    slot_maps = []
    for c in range(N_CORES):
        s, x, nn, _, _ = per_core[c]
        slots = all_slots[c]
        gidx_all = np.zeros(nt_tot * 128, np.int16)
        srel_all = np.full((nt_tot, 128), -1.0, np.float32)
        cnt1 = np.ones((128, n_slots), np.float32)
        node_bnds = [sl[0] for sl in slots] + [nn]
        edge_bnds = np.searchsorted(s, node_bnds)
        for g, (base_n, width, _, _) in enumerate(slots):
            t_l, t_h = t_l_arr[g], t_h_arr[g]
            kl, kh = t_l * 128, t_h * 128
            a, b = edge_bnds[g], edge_bnds[g + 1]
            sg = s[a:b]
            xg = x[a:b]
            m = xg < SPLIT
            xl = xg[m]
            xh = xg[~m] - SPLIT
            sl_ = sg[m] - base_n
            sh_ = sg[~m] - base_n
            # sort each run by source row: the one-hot matmul is order-
            # invariant within a slot, and address-sorted gather descriptors
            # get HBM row-buffer locality (duplicates become adjacent)
            ol = np.argsort(xl, kind="stable")
            xl, sl_ = xl[ol], sl_[ol]
            oh_ = np.argsort(xh, kind="stable")
            xh, sh_ = xh[oh_], sh_[oh_]
            base_l = int(cum_lt[g]) * 128
            base_h = rows_l + int(cum_ht[g]) * 128
            gidx_all[base_l : base_l + len(xl)] = xl.astype(np.int16)
            gidx_all[base_h : base_h + len(xh)] = xh.astype(np.int16)
            srl = np.full(kl, -1.0, np.float32)
            srl[: len(sl_)] = sl_
            srel_all[cum_lt[g] : cum_lt[g] + t_l] = srl.reshape(t_l, 128)
            srh = np.full(kh, -1.0, np.float32)
            srh[: len(sh_)] = sh_
            srel_all[nt_l + cum_ht[g] : nt_l + cum_ht[g] + t_h] = (
                srh.reshape(t_h, 128)
            )
            abs_base = bounds[c] + base_n
            cnt1[:width, g] = 1.0 + deg[abs_base : abs_base + width]

        gidx_w = gidx_all.reshape(-1, 16).T
        if GIDXPE:
            gidx_w = np.ascontiguousarray(gidx_w.astype(np.float32))
        else:
            if not GIDX16:
                gidx_w = np.tile(gidx_w, (8, 1))
            gidx_w = np.ascontiguousarray(gidx_w)
        srel_mat = np.ascontiguousarray(srel_all.T).astype(bf16)
        imap = {
            "featb": featb,
            "srel": srel_mat,
            "cnt1": cnt1,
        }
        if GIDXPE:
            imap["gidxf"] = gidx_w
            imap["repmat"] = np.ascontiguousarray(
                (np.arange(128)[None, :] % 16 == np.arange(16)[:, None])
                .astype(np.float32)
            )
        else:
            imap["gidx"] = gidx_w
        in_maps.append(imap)
        slot_maps.append(
            [(bounds[c] + sl[0], sl[1]) for sl in slots]
        )
    return t_l_arr, t_h_arr, in_maps, slot_maps


def kernel(features, neighbor_idx, segment_ids):
    global LAST_NC
    t_l_arr, t_h_arr, in_maps, slot_maps = _preprocess(
        features, neighbor_idx, segment_ids
    )

    key = (t_l_arr, t_h_arr)
    if key not in _PROGRAM_CACHE:
        _PROGRAM_CACHE[key] = _build_program(t_l_arr, t_h_arr)
    nc = _PROGRAM_CACHE[key]
    LAST_NC = nc

    try:
        res = bass_utils.run_bass_kernel_spmd(
            nc, in_maps, core_ids=list(range(N_CORES))
        )
    except Exception:
        # transient axon/device hiccups (e.g. recovering from a prior wedge)
        # have been observed to clear after a short pause
        import time

        time.sleep(20)
        res = bass_utils.run_bass_kernel_spmd(
            nc, in_maps, core_ids=list(range(N_CORES))
        )

    out = np.empty((N_NODES, D), np.float32)
    for c in range(N_CORES):
        oc = res.results[c]["out"].astype(np.float32)
        for g, (abs_base, width) in enumerate(slot_maps[c]):
            out[abs_base : abs_base + width] = oc[g * 128 : g * 128 + width]
    return out
